# revision 2
# baseline (speedup 1.0000x reference)
"""Trainium2 Bass kernel for CandidateFinder (retrieval_knn).

Math: for each (batch, query row), candidates = the K_MAX=64 smallest key
indices whose 32-dim sign pattern matches the query's in either dim-group
(dims 0:32 or 32:64), ascending, padded with -1.

Structure: a fast SCREEN kernel computes an exact "any match anywhere"
flag (zero false negatives: exact (x>0) quantize, exact fp32 dots,
threshold between the best non-match 7.5 and the match value 8.0) plus
the all-(-1) output.  The host inspects the device-computed flag and only
if a match exists launches the EXACT kernel (the previous full
implementation, lazily compiled) to rewrite the output.  With random
inputs a 32-bit sign collision has probability ~2^-32 per pair, so the
screen path is the only one that runs; the exact path keeps kernel()
correct for any input.

Screen kernel per core (core 2b+half handles batch b, query half):
  - inputs dim-major: queries duplicated onto the upper 64 partitions,
    keys split into two 1024-column halves stacked on the partition axis,
    so four K=32 matmuls run concurrently in PE row-groups
  - sign-quantize to {+0.5,-0.5} bf16 on DVE (x>0 exactly as reference)
  - all-pairs group dots (match <=> dot == 8); per-tile detection split
    DVE (row-max) / ACT (relu-sum with accumulator), threshold 7.9
  - flag via ones-matmul partition reduce; no device branch at all
"""

import numpy as np

import concourse.bacc as bacc
import concourse.mybir as mybir
from concourse.tile import TileContext
from concourse import bass_utils

B, L, D = 4, 2048, 64
HALF = 1024          # query rows per core
N_CORES = 8
K_MAX = 64
QT = HALF // 128     # 8 query slabs per core
THRESH = 7.9         # between 7.5 (best non-match) and 8.0 (match)
SENT = 4096.0        # sentinel > any index (exact kernel)

f32 = mybir.dt.float32
bf16 = mybir.dt.bfloat16
i32 = mybir.dt.int32
u32 = mybir.dt.uint32
Alu = mybir.AluOpType
Ax = mybir.AxisListType
AF = mybir.ActivationFunctionType

_CACHE = {}


def _build_screen():
    nc = bacc.Bacc("TRN2", target_bir_lowering=False,
                   enable_partition_id=False)
    # qd[g*32+d + 64*dup, p] = q[p, g*32+d]   (dims-major, dup to 64:128)
    qd = nc.dram_tensor("qd", [128, HALF], f32, kind="ExternalInput")
    # kh[d + 64*h, j] = k[h*1024 + j, d]      (key halves stacked)
    kh = nc.dram_tensor("kh", [128, 1024], f32, kind="ExternalInput")
    out = nc.dram_tensor("out", [HALF, K_MAX], i32, kind="ExternalOutput")
    flag = nc.dram_tensor("flag", [1, 1], i32, kind="ExternalOutput")
    out_pt = out[:].rearrange("(p t) c -> p (t c)", p=128)

    with TileContext(nc) as tc:
        with tc.tile_pool(name="sb", bufs=1) as sb, \
             tc.tile_pool(name="scr", bufs=1) as scrp, \
             tc.tile_pool(name="ps", bufs=4, space="PSUM") as ps:

            ksb = sb.tile([128, 1024], f32)
            qsb = sb.tile([128, HALF], f32)
            # all input DMA on the sync HWDGE ring, k first; the scalar
            # engine queue stays empty so its activation-table load runs
            # at t~0, overlapped with the DMA wait
            nc.sync.dma_start(ksb[:, 0:512], kh[:, 0:512])
            nc.sync.dma_start(qsb[:, 0:512], qd[:, 0:512])
            nc.sync.dma_start(ksb[:, 512:1024], kh[:, 512:1024])
            nc.sync.dma_start(qsb[:, 512:1024], qd[:, 512:1024])

            # early fast-path output: all -1 (row permutation irrelevant)
            out_sb = sb.tile([128, QT * K_MAX], i32)
            nc.gpsimd.memset(out_sb, -1)
            nc.sync.dma_start(out_pt, out_sb)

            # sign-quantize (x>0 -> +0.5 else -0.5, bf16), chunked so it
            # starts as soon as each DMA chunk lands
            sk = sb.tile([128, 1024], bf16)
            sq = sb.tile([128, HALF], bf16)
            nc.vector.tensor_scalar(sk[:, 0:512], ksb[:, 0:512], 0.0, 0.5,
                                    op0=Alu.is_gt, op1=Alu.subtract)
            nc.vector.tensor_scalar(sq[:, 0:512], qsb[:, 0:512], 0.0, 0.5,
                                    op0=Alu.is_gt, op1=Alu.subtract)
            nc.vector.tensor_scalar(sk[:, 512:1024], ksb[:, 512:1024],
                                    0.0, 0.5,
                                    op0=Alu.is_gt, op1=Alu.subtract)
            nc.vector.tensor_scalar(sq[:, 512:1024], qsb[:, 512:1024],
                                    0.0, 0.5,
                                    op0=Alu.is_gt, op1=Alu.subtract)

            # ---- all-pairs dots, 4 concurrent K=32 matmuls per iter ----
            # rstat col semantics: >= THRESH iff any match (DVE max cols
            # top out at 8; ACT relu-sum cols are 0 or >= 8)
            rstat = sb.tile([128, 32], f32)
            rbias = sb.tile([128, 1], f32)
            nc.vector.memset(rbias, -80.0 * THRESH)
            scr = scrp.tile([128, 1024], bf16)
            it = 0
            for j in range(2):          # key column block within half
                kc = slice(j * 512, (j + 1) * 512)
                for s in range(QT):     # query slab
                    qc = slice(s * 128, (s + 1) * 128)
                    pA = ps.tile([128, 1024], f32, tag="p")
                    pB = ps.tile([128, 1024], f32, tag="p")
                    # lower key half (partitions 0:64)
                    nc.tensor.matmul(pA[:, 0:512], lhsT=sq[0:32, qc],
                                     rhs=sk[0:32, kc], start=True,
                                     stop=True, tile_position=(0, 0))
                    nc.tensor.matmul(pA[:, 512:1024], lhsT=sq[32:64, qc],
                                     rhs=sk[32:64, kc], start=True,
                                     stop=True, tile_position=(32, 0))
                    # upper key half (partitions 64:128)
                    nc.tensor.matmul(pB[:, 0:512], lhsT=sq[64:96, qc],
                                     rhs=sk[64:96, kc], start=True,
                                     stop=True, tile_position=(64, 0))
                    nc.tensor.matmul(pB[:, 512:1024], lhsT=sq[96:128, qc],
                                     rhs=sk[96:128, kc], start=True,
                                     stop=True, tile_position=(96, 0))
                    # detection split: DVE row-max / ACT relu-accum
                    nc.vector.tensor_reduce(rstat[:, 2 * it:2 * it + 1],
                                            pA, axis=Ax.X, op=Alu.max)
                    nc.scalar.activation(
                        scr, pB, AF.Relu, bias=rbias[:, 0:1], scale=80.0,
                        accum_out=rstat[:, 2 * it + 1:2 * it + 2])
                    it += 1

            # ---- scalar any-match flag ----
            ones = sb.tile([128, 1], f32)
            nc.vector.memset(ones, 1.0)
            sr = sb.tile([128, 1], f32)
            nc.vector.tensor_reduce(sr, rstat, axis=Ax.X, op=Alu.max)
            srf = sb.tile([128, 1], f32)
            nc.vector.tensor_scalar(srf, sr, THRESH, None, op0=Alu.is_ge)
            fps = ps.tile([1, 1], f32, tag="p")
            nc.tensor.matmul(fps, lhsT=ones, rhs=srf, start=True, stop=True)
            fsb = sb.tile([1, 1], i32)
            nc.vector.tensor_scalar(fsb, fps[0:1, 0:1], 0.5, None,
                                    op0=Alu.is_ge)
            nc.sync.dma_start(flag[:], fsb)

    nc.compile()
    return nc


def get_nc():
    if "nc" not in _CACHE:
        _CACHE["nc"] = _build_screen()
    return _CACHE["nc"]


def make_in_maps(query_up, key_up):
    """Pure layout transforms (transpose/stack/duplicate) per core."""
    query_up = np.asarray(query_up, dtype=np.float32)
    key_up = np.asarray(key_up, dtype=np.float32)
    in_maps = []
    for c in range(N_CORES):
        b, half = c // 2, c % 2
        qT = query_up[b, half * HALF:(half + 1) * HALF].T   # [64, 1024]
        qd = np.ascontiguousarray(np.concatenate([qT, qT], axis=0))
        k = key_up[b]                                       # [2048, 64]
        kh = np.ascontiguousarray(
            np.concatenate([k[0:1024].T, k[1024:2048].T], axis=0))
        in_maps.append({"qd": qd, "kh": kh})
    return in_maps


# ---------------------------------------------------------------------------
# Exact kernel (previous full implementation) -- only compiled and run if the
# screen flag fires, i.e. some query/key pair shares a 32-bit sign pattern.
# ---------------------------------------------------------------------------

MATCH_DOT = 8.0


def _build_exact():
    nc = bacc.Bacc("TRN2", target_bir_lowering=False,
                   enable_partition_id=False)
    # qt4[h*64+d, pair*128+p] = q[p*8 + 2*pair + h, d]
    qt4 = nc.dram_tensor("qt4", [128, HALF // 2], f32, kind="ExternalInput")
    # kt4[dup*64+d, j] = k[j, d]
    kt4 = nc.dram_tensor("kt4", [128, L], f32, kind="ExternalInput")
    out = nc.dram_tensor("out", [HALF, K_MAX], i32, kind="ExternalOutput")
    out_pt = out[:].rearrange("(p t) c -> p (t c)", p=128)

    with TileContext(nc) as tc:
        with tc.tile_pool(name="sb", bufs=1) as sb, \
             tc.tile_pool(name="sb2", bufs=3) as sb2, \
             tc.tile_pool(name="ps", bufs=2, space="PSUM") as ps:

            qsb = sb.tile([128, HALF // 2], f32)
            ksb = sb.tile([128, L], f32)
            sqT4 = sb.tile([128, HALF // 2], bf16)
            skT4 = sb.tile([128, L], bf16)
            nc.default_dma_engine.dma_start(ksb[:, 0:1024], kt4[:, 0:1024])
            nc.scalar.dma_start(ksb[:, 1024:2048], kt4[:, 1024:2048])
            nc.default_dma_engine.dma_start(qsb, qt4[:, :])
            nc.vector.tensor_scalar(skT4, ksb, 0.0, 0.5,
                                    op0=Alu.is_gt, op1=Alu.subtract)
            nc.vector.tensor_scalar(sqT4, qsb, 0.0, 0.5,
                                    op0=Alu.is_gt, op1=Alu.subtract)

            out_sb = sb.tile([128, QT * K_MAX], i32)
            nc.gpsimd.memset(out_sb, -1)

            c2i = sb.tile([128, L], i32)   # SENT - j (key j = column)
            nc.gpsimd.iota(c2i, pattern=[[-1, L]], base=int(SENT),
                           channel_multiplier=0)
            c2f = sb.tile([128, L], f32)
            nc.gpsimd.tensor_copy(c2f, c2i)
            negone = sb.tile([128, K_MAX], f32)
            nc.vector.memset(negone, -1.0)
            for t in range(QT):
                base = (t % 2) * 64
                qc = slice((t // 2) * 128, (t // 2) * 128 + 128)
                lhs0 = sqT4[base:base + 32, qc]
                lhs1 = sqT4[base + 32:base + 64, qc]
                val = sb.tile([128, L], f32, tag="val")
                for h in range(2):
                    p0 = ps.tile([128, 1024], f32, tag="g0")
                    p1 = ps.tile([128, 1024], f32, tag="g1")
                    for sblk in range(2):
                        kc = slice(h * 1024 + sblk * 512,
                                   h * 1024 + (sblk + 1) * 512)
                        sl = slice(sblk * 512, (sblk + 1) * 512)
                        nc.tensor.matmul(p0[:, sl], lhsT=lhs0,
                                         rhs=skT4[base:base + 32, kc],
                                         start=True, stop=True,
                                         tile_position=(base, 0))
                        nc.tensor.matmul(p1[:, sl], lhsT=lhs1,
                                         rhs=skT4[base + 32:base + 64, kc],
                                         start=True, stop=True,
                                         tile_position=(base + 32, 0))
                    hsl = slice(h * 1024, (h + 1) * 1024)
                    m0 = sb2.tile([128, 1024], f32, tag="m0")
                    nc.vector.tensor_scalar(m0, p0, THRESH,
                                            None, op0=Alu.is_ge)
                    m1 = sb2.tile([128, 1024], f32, tag="m1")
                    nc.vector.scalar_tensor_tensor(
                        m1, in0=p1, scalar=THRESH, in1=m0,
                        op0=Alu.is_ge, op1=Alu.max)
                    # val = m1 ? -(j) : -SENT  ==  m1*(SENT-j) - SENT
                    nc.vector.tensor_tensor(
                        out=val[:, hsl], in0=m1, in1=c2f[:, hsl],
                        op=Alu.mult)
                    nc.vector.tensor_scalar_add(val[:, hsl], val[:, hsl],
                                                -SENT)
                # 64 smallest j == 64 largest of val, descending
                no = sb.tile([128, K_MAX], f32, tag="no")
                for it8 in range(8):
                    osl = slice(it8 * 8, (it8 + 1) * 8)
                    nc.vector.max(out=no[:, osl], in_=val)
                    nc.vector.match_replace(
                        out=val, in_to_replace=no[:, osl],
                        in_values=val, imm_value=-SENT)
                jv = sb.tile([128, K_MAX], f32, tag="jv")
                nc.vector.tensor_scalar_mul(jv, no, -1.0)  # j or SENT
                msk = sb.tile([128, K_MAX], u32, tag="msk")
                nc.vector.tensor_scalar(msk, jv, 2048.5, None,
                                        op0=Alu.is_ge)
                nc.vector.copy_predicated(jv, msk, negone)
                nc.vector.tensor_copy(
                    out_sb[:, t * K_MAX:(t + 1) * K_MAX], jv)

            nc.default_dma_engine.dma_start(out_pt, out_sb)

    nc.compile()
    return nc


def get_nc_exact():
    if "nc_exact" not in _CACHE:
        _CACHE["nc_exact"] = _build_exact()
    return _CACHE["nc_exact"]


def make_in_maps_exact(query_up, key_up):
    query_up = np.asarray(query_up, dtype=np.float32)
    key_up = np.asarray(key_up, dtype=np.float32)
    in_maps = []
    for c in range(N_CORES):
        b, half = c // 2, c % 2
        q = query_up[b, half * HALF:(half + 1) * HALF]       # [1024, 64]
        qt4 = np.ascontiguousarray(
            q.reshape(128, 4, 2, D).transpose(2, 3, 1, 0).reshape(
                128, HALF // 2))
        kT = key_up[b].T                                     # [64, 2048]
        kt4 = np.ascontiguousarray(np.concatenate([kT, kT], axis=0))
        in_maps.append({"qt4": qt4, "kt4": kt4})
    return in_maps


def kernel(query_up, key_up, head_idx=None, **_ignored):
    nc = get_nc()
    in_maps = make_in_maps(query_up, key_up)
    res = bass_utils.run_bass_kernel_spmd(
        nc, in_maps, core_ids=list(range(N_CORES)))
    full = np.empty((B, L, K_MAX), dtype=np.int32)
    if any(int(res.results[c]["flag"][0, 0]) for c in range(N_CORES)):
        # rare: some pair shares a full 32-bit sign pattern -> exact kernel
        nce = get_nc_exact()
        res_e = bass_utils.run_bass_kernel_spmd(
            nce, make_in_maps_exact(query_up, key_up),
            core_ids=list(range(N_CORES)))
        for c in range(N_CORES):
            b, half = c // 2, c % 2
            full[b, half * HALF:(half + 1) * HALF] = res_e.results[c]["out"]
    else:
        for c in range(N_CORES):
            b, half = c // 2, c % 2
            full[b, half * HALF:(half + 1) * HALF] = res.results[c]["out"]
    return full
